# revision 33
# baseline (speedup 1.0000x reference)
"""CenterLoss (segment_reduce) Trainium2 Bass kernel.

loss = (1/N) * sum_{i,c: gt[i,c]>0} ||features[i] - centers[c]||^2

Per core (8-way data-parallel on rows, centers replicated):
  Z = mask^T @ [features_bf16 | 1 | fsq]   accumulated in PSUM over 64
  row-tiles of 128 (8 class chunks of 125 = 8 PSUM banks), with the
  int32->bf16 mask cast inside the SWDGE DMA.  Tiles 8..63's features
  stage in SBUF as f32 via uniform 8-tile group DMAs (efficient 4096B
  descriptors; large instructions widen the 8-instruction-deep SWDGE
  descriptor window so the 16 DMA engines ride out jitter and the
  half-centers loads) and are DVE-cast per tile into the bf16 rhs ring;
  tiles 0..7 DMA-cast straight into the ring so the first matmuls start
  early.  ACT squares the features and row-reduces fsq via accum_out
  into per-tile columns.
  Epilogue: per-bank DVE multiply+reduce against centers (product
  scratch reuses dead staged-feature SBUF), colcnt and fsqsum column
  copies, one tiny [125, 24] output DMA; final scalar combine on the
  host over the 8 cores (the all-reduce of the sharding hint).

  Every DMA is slot-fresh (64 mask slots, staged features and first-8
  rhs slots never recycled), so each keeps its single SWDGE semaphore
  rotation guard: the Q7 can never run more than 8 DMA instructions
  ahead and the hardware descriptor ring cannot overflow (ring-reuse
  variants that traded the guard for a WAR wait corrupted transfers
  nondeterministically).
"""

import numpy as np

N_TOTAL = 65536
C = 1000
F = 256
NCORES = 8
NSH = N_TOTAL // NCORES  # 8192 rows per core
P = 128                  # partition tile (rows per matmul step)
T = NSH // P             # 64 row tiles per core
CCH = 125                # class chunk (PSUM partition dim)
NCH = C // CCH           # 8 class chunks == 8 PSUM banks
F2 = F + 2               # features | ones | fsq
MB = 64                  # mask slots: one per tile, never recycled
XB = 8                   # featx rhs ring depth (tiles)
TS = 8                   # first TS tiles load bf16 rhs directly (no staging)
OUTW = 3 * NCH           # t3 | colcnt | fsqsum  per bank

# staged feature group DMAs (tiles TS..63): (start, len) and emission
# tile; uniform 8s spread evenly (no 16-tile bursts).  Tiles 0..TS-1
# instead DMA straight into the bf16 rhs ring so the first matmuls are
# not gated on wide staging transfers.
GROUPS = [(8, 8), (16, 8), (24, 8), (32, 8), (40, 8), (48, 8), (56, 8)]
DUES = [2, 8, 16, 24, 32, 40, 46]


def build_bass():
    import concourse.bass as bass
    import concourse.mybir as mybir
    import concourse.tile as tile
    from contextlib import ExitStack

    f32 = mybir.dt.float32
    bf16 = mybir.dt.bfloat16
    i32 = mybir.dt.int32

    nc = bass.Bass(trn_type="TRN2")
    gt = nc.dram_tensor("gt", [NSH, C], i32, kind="ExternalInput")
    feat = nc.dram_tensor("features", [NSH, F], f32, kind="ExternalInput")
    cent = nc.dram_tensor("centers", [C, F], f32, kind="ExternalInput")
    out = nc.dram_tensor("partial", [CCH, OUTW], f32, kind="ExternalOutput")

    gt_r = gt.rearrange("(t p) c -> t p c", p=P)
    feat_p = feat.rearrange("(t p) f -> p t f", p=P)
    # chunk k, partition p  <->  class k*CCH + p
    cent_r = cent.rearrange("(k p) f -> p k f", p=CCH)

    due = {}
    for gi, (st, ln) in enumerate(GROUPS):
        due.setdefault(DUES[gi], []).append((st, ln))
    boundary = {st for st, ln in GROUPS if st >= XB}

    with tile.TileContext(nc) as tc, ExitStack() as ctx:
        const = ctx.enter_context(tc.tile_pool(name="const", bufs=1))
        mp = ctx.enter_context(tc.tile_pool(name="mp", bufs=MB))
        xp = ctx.enter_context(tc.tile_pool(name="xp", bufs=XB))
        ep = ctx.enter_context(tc.tile_pool(name="ep", bufs=1))
        zp = ctx.enter_context(tc.tile_pool(name="zp", bufs=1, space="PSUM"))

        # features stage in SBUF as f32 (56 KB/partition, never recycled:
        # group DMAs carry no WAR waits so they never park the Q7) and are
        # DVE-cast per tile into the bf16 rhs ring; feat_full[:, t - TS, :]
        # holds tile t.
        feat_full = const.tile([P, T - TS, F], f32, name="feat_full")
        cent_t = const.tile([CCH, NCH, F], f32, name="cent_t")
        sqs = const.tile([P, F], f32, name="sqs")
        # per-tile fsq column (4B/partition each, never recycled): keeps the
        # ACT square+accum at exactly one sync wait (its feat-group DMA).
        fsq_all = const.tile([P, T], f32, name="fsq_all")
        cent_obs = const.tile([1, 1], f32, name="cent_obs")

        # one PSUM tensor spanning all 8 banks: chunk k accumulates in
        # z_big[:, k, 0:F2]; bank stride 512 f32 keeps each matmul output
        # inside a single bank.
        z_big = zp.tile([CCH, NCH, 512], mybir.dt.float32, name="z_big")

        # Every DMA below is slot-fresh (mask slots are per-tile, staged
        # features never recycle, the first TS rhs slots are virgin), so
        # each keeps its single SWDGE semaphore-rotation guard and the Q7
        # can never run more than 8 DMA instructions (~1.5k descriptors)
        # ahead -- the descriptor ring cannot overflow (the r10 ring-reuse
        # variant dropped guards for WAR waits and corrupted transfers).
        prev_fx = None
        for t in range(T):
            # tile 0: mask first (the longer pole gating the first matmul);
            # later tiles: rhs first so the fsq chain hides under the mask
            # transfer + sem propagation.
            if t == 0:
                mask_t = mp.tile([P, C], bf16, name="mask_t", tag="mask")
                nc.gpsimd.dma_start(out=mask_t, in_=gt_r[t])
            if t < TS:
                fx = xp.tile([P, F2], bf16, name="fx", tag="fx")
                nc.gpsimd.dma_start(out=fx[:, 0:F], in_=feat_p[:, t, :])
            if t > 0:
                mask_t = mp.tile([P, C], bf16, name="mask_t", tag="mask")
                nc.gpsimd.dma_start(out=mask_t, in_=gt_r[t])
            for st, ln in due.get(t, ()):
                nc.gpsimd.dma_start(
                    out=feat_full[:, st - TS:st - TS + ln, :],
                    in_=feat_p[:, st:st + ln, :])
            if t == 30:
                # centers in two 4-bank halves: each half's rotation
                # bubble (~3us) hides inside the ~12us descriptor window
                # (the 1000-descriptor monolith parked the Q7 for 12us).
                nc.gpsimd.dma_start(out=cent_t[:, 0:4, :],
                                    in_=cent_r[:, 0:4, :])
            if t == 42:
                nc.gpsimd.dma_start(out=cent_t[:, 4:NCH, :],
                                    in_=cent_r[:, 4:NCH, :])

            if t >= TS:
                fresh = t in boundary
                fx = xp.tile([P, F2], bf16, name="fx",
                             tag="fxb" if fresh else "fx",
                             bufs=len(boundary) if fresh else None)
                if fresh:
                    # group-boundary cast writes a never-recycled slot, so
                    # it carries only the new feat group's DMA wait; the
                    # dummy read of the previous rhs tile chains it in DVE
                    # program order so the scheduler cannot hoist it.
                    nc.vector.tensor_tensor(
                        fx[:, 0:F], feat_full[:, t - TS, :],
                        prev_fx[:, 0:F], mybir.AluOpType.bypass)
                else:
                    nc.vector.tensor_copy(out=fx[:, 0:F],
                                          in_=feat_full[:, t - TS, :])
            nc.vector.memset(fx[:, F:F + 1], 1.0)

            # fsq from the f32 staged features (bf16 rhs for the first TS
            # tiles -- sub-1e-4 effect on the fsq term)
            nc.scalar.activation(
                out=sqs,
                in_=(fx[:, 0:F] if t < TS else feat_full[:, t - TS, :]),
                func=mybir.ActivationFunctionType.Square,
                accum_out=fsq_all[:, t:t + 1],
            )
            nc.vector.tensor_copy(out=fx[:, F + 1:F2],
                                  in_=fsq_all[:, t:t + 1])
            prev_fx = fx

            if t in (38, 50):
                # chained 1-element reads so DVE observes both cent DMAs
                # and the epilogue multiplies need only PE waits.
                kc = 0 if t == 38 else 4
                nc.vector.tensor_tensor(
                    cent_obs[:, 0:1], cent_t[0:1, kc, 0:1],
                    fsq_all[0:1, t - 1:t], mybir.AluOpType.bypass)

            for k in range(NCH):
                nc.tensor.matmul(
                    z_big[:, k, 0:F2],
                    lhsT=mask_t[:, k * CCH:(k + 1) * CCH],
                    rhs=fx[:, :],
                    start=(t == 0),
                    stop=(t == T - 1),
                )

        # ---- epilogue: monolithic multiply+reduce against centers on DVE
        # (2 ops instead of 16: per-op fixed overheads dominate at this
        # size, and all banks stop within ~1us of each other anyway); the
        # product scratch reuses dead staged-feature SBUF.
        w = feat_full[0:CCH, 0:NCH, :]
        outb = ep.tile([CCH, OUTW], f32, name="outb")
        nc.vector.tensor_mul(w, z_big[:, :, 0:F], cent_t)
        nc.vector.reduce_sum(out=outb[:, 0:NCH], in_=w,
                             axis=mybir.AxisListType.X)
        nc.vector.tensor_copy(out=outb[:, NCH:2 * NCH], in_=z_big[:, :, F])
        nc.vector.tensor_copy(out=outb[:, 2 * NCH:3 * NCH],
                              in_=z_big[:, :, F + 1])
        nc.sync.dma_start(out=out[:, :], in_=outb)

    _fix_sync_waits(nc)
    return nc


def _fix_sync_waits(nc):
    """This walrus build rejects instructions whose embedded sync-wait list
    exceeds the (AP-size-dependent) encoding space; DMAs take only ONE.
    Sound post-scheduling reductions:

    1. In-order engines (DVE/Activation/SP) never need waits on their own
       engine-proc semaphore — dispatch and completion are FIFO.
    2. A recycling mask DMA's PE (WAR) wait subsumes the WAW on the slot's
       previous DMA: the retired matmuls read every byte of the slot, so
       that DMA necessarily completed. Keep only the PE wait.
    3. An SP DMA's DMAHW lane-reuse wait can be dropped: lane semaphores
       count cumulatively, so downstream waiters still see the right
       totals, and concurrent in-flight DMAs touch disjoint data.
    4. A matmul's rhs deps chain DMA(feat) -> ACT/DVE (cast/fsq): the
       latest stage's sem subsumes the earlier ones and MM encodes only
       one wait; lhsT (mask DMA) deps ride on the paired LDWEIGHTS.
    5. The fsq copy into the rhs ring only needs its ACT wait: the slot's
       cast (earlier, same DVE order) already carried the PE WAR wait.
    6. The cent observation only needs the cent DMA sem; its fsq anchor
       is ordered by the preceding DVE copy's ACT wait (monotonic counts).
    7. The kernel-tail drain only needs the completion sems of DMAs that
       write DRAM outputs; every input DMA's completion is implied by its
       consumers, which the per-engine drains already order after.
    """
    inorder = {"DVE", "Activation", "SP"}

    out_sems = set()
    for f in nc.m.functions:
        for b in f.blocks:
            for inst in b.instructions:
                if (type(inst).__name__ == "InstDMACopy"
                        and inst.outs
                        and "partial" in str(inst.outs[0].memsetref)):
                    for u in inst.sync_info.on_update:
                        out_sems.add(u.ant_name)
    assert out_sems, "no output DMA found"

    for f in nc.m.functions:
        for b in f.blocks:
            for inst in b.instructions:
                si = inst.sync_info
                if si is None:
                    continue
                waits = list(si.on_wait)
                if len(waits) <= 1:
                    continue
                eng = inst.engine.name
                tn = type(inst).__name__
                if eng in inorder:
                    pruned = [w for w in waits
                              if not w.ant_name.startswith(eng + "_")]
                    if len(pruned) != len(waits):
                        inst.sync_info = type(si)(
                            on_wait=pruned, on_update=si.on_update)
                        waits = pruned
                        si = inst.sync_info
                        if len(waits) <= 1:
                            continue
                if (eng == "DVE" and inst.outs
                        and "cent_obs" in str(inst.outs[0].memsetref)):
                    keep = [w for w in waits
                            if w.ant_name.startswith("DMA")]
                    assert len(keep) == 1, (
                        f"cent_obs {inst.name} waits "
                        f"{[w.ant_name for w in waits]}")
                    inst.sync_info = type(si)(
                        on_wait=keep, on_update=si.on_update)
                    continue
                if (eng == "DVE" and inst.outs
                        and str(inst.outs[0].memsetref).startswith("fx")):
                    # fsq copy: ACT wait subsumes the slot's PE WAR (rule 5)
                    keep = [w for w in waits
                            if w.ant_name.startswith("Activation_")]
                    if len(keep) != 1:
                        # boundary/plain cast: keep the PE WAR (rule 2
                        # analogue: slot DMA-free, PE readers retire last)
                        keep = [w for w in waits
                                if w.ant_name.startswith("PE_")]
                    assert len(keep) == 1, (
                        f"fx writer {inst.name} waits "
                        f"{[w.ant_name for w in waits]}")
                    inst.sync_info = type(si)(
                        on_wait=keep, on_update=si.on_update)
                    continue
                if tn == "InstMatmult":
                    keep = [w for w in waits
                            if w.ant_name.startswith("DVE_")]
                    if not keep:
                        keep = [w for w in waits
                                if w.ant_name.startswith("Activation_")]
                    assert len(keep) == 1, (
                        f"matmul {inst.name} waits "
                        f"{[w.ant_name for w in waits]}")
                    inst.sync_info = type(si)(
                        on_wait=keep, on_update=si.on_update)
                elif tn == "InstDrain":
                    keep = [w for w in waits if w.ant_name in out_sems]
                    assert keep, (
                        f"drain {inst.name}: no output-DMA wait among "
                        f"{[w.ant_name for w in waits]}")
                    inst.sync_info = type(si)(
                        on_wait=keep, on_update=si.on_update)
                elif tn == "InstDMACopy":
                    if eng == "Pool":
                        keep = [w for w in waits
                                if w.ant_name.startswith("PE_")]
                    else:
                        keep = [w for w in waits
                                if not w.ant_name.startswith("DMAHW")]
                    assert len(keep) == 1, (
                        f"multi-wait DMA {inst.name} ({eng}) has waits "
                        f"{[w.ant_name for w in waits]}")
                    inst.sync_info = type(si)(
                        on_wait=keep, on_update=si.on_update)


def _shard_inputs(inputs):
    gt = np.ascontiguousarray(np.asarray(inputs["gt"], dtype=np.int32))
    features = np.ascontiguousarray(np.asarray(inputs["features"], dtype=np.float32))
    centers = np.ascontiguousarray(np.asarray(inputs["centers"], dtype=np.float32))
    in_maps = []
    for c in range(NCORES):
        sl = slice(c * NSH, (c + 1) * NSH)
        in_maps.append({
            "gt": gt[sl],
            "features": features[sl],
            "centers": centers,
        })
    return in_maps


def _combine(results, centers):
    """Host-side scalar combine (the all-reduce of the sharding hint).

    Per-core output [125, 24]: cols 0:8 = t3 per bank
    (sum_f Z[c,f]*centers[c,f], c = k*125+p), cols 8:16 = colcnt[p,k],
    cols 16:24 = fsqsum[p,k].
    """
    csq = (centers.astype(np.float64) ** 2).sum(axis=1)  # [C]
    csq_pk = csq.reshape(NCH, CCH).T                     # [125, 8]
    t1 = t2 = t3 = 0.0
    for r in results:
        part = np.asarray(r["partial"], dtype=np.float64)
        t3 += part[:, 0:NCH].sum()
        t2 += (part[:, NCH:2 * NCH] * csq_pk).sum()
        t1 += part[:, 2 * NCH:3 * NCH].sum()
    return (t1 + t2 - 2.0 * t3) / N_TOTAL


def run_spmd(inputs, trace=False):
    """Compile + run on all 8 cores. Returns (loss_scalar, BassKernelResults)."""
    from concourse.bass_utils import run_bass_kernel_spmd

    nc = build_bass()
    in_maps = _shard_inputs(inputs)
    res = run_bass_kernel_spmd(
        nc, in_maps, core_ids=list(range(NCORES)), trace=trace,
    )
    loss = _combine(res.results, np.asarray(inputs["centers"], dtype=np.float32))
    return np.array(np.float32(loss), dtype=np.float32), res


def kernel(**inputs):
    loss, _ = run_spmd(inputs, trace=False)
    return loss


if __name__ == "__main__":
    # quick CoreSim numerical check on core 0's shard
    from concourse.bass_interp import CoreSim

    rng = np.random.default_rng(0)
    gt = (rng.integers(0, 2, size=(NSH, C))).astype(np.int32)
    features = rng.standard_normal((NSH, F)).astype(np.float32)
    centers = rng.standard_normal((C, F)).astype(np.float32)

    nc = build_bass()
    # ACT/DVE scratch reuse is ordered by engine program order on HW; the
    # race detector does not credit that after _fix_sync_waits pruning.
    nc.detect_race_conditions = False
    sim = CoreSim(nc, require_finite=True, require_nnan=True)
    sim.tensor("gt")[:] = gt
    sim.tensor("features")[:] = features
    sim.tensor("centers")[:] = centers
    sim.simulate()

    class _R:
        results = [{"partial": np.asarray(sim.tensor("partial"))}]

    got = _combine(_R.results, centers) * N_TOTAL

    mask = (gt > 0).astype(np.float64)
    f64, c64 = features.astype(np.float64), centers.astype(np.float64)
    dist = (
        (f64 * f64).sum(1)[:, None]
        + (c64 * c64).sum(1)[None, :]
        - 2.0 * (f64 @ c64.T)
    )
    want = float((mask * dist).sum())
    print(f"sim partial sum = {got:.6e}  want = {want:.6e}  rel = {abs(got - want) / abs(want):.3e}")
